# revision 19
# baseline (speedup 1.0000x reference)
"""Trainium2 Bass kernel for nn_Memory_Network (retrieval kNN).

reference semantics:
    q = query / ||query||_2                  [128, 512]
    scores = q @ spatial_key.T               [128, 262144]
    topk_score, topk_index = top_k(scores, 256)
    topk_feat = color_value[topk_index]      [128, 256, 512]
    returns (topk_feat, topk_score, topk_index)

Sharding: the memory bank is split along mem_size across 8 cores (32768 rows
each).  Each core streams its (host-transposed, fp16) key shard through the
PE — queries live on the PSUM partitions, scores accumulate in fp32.

Candidate extraction packs the column index into the score's low mantissa
bits so a single DVE MAX both selects and identifies candidates: the ACT
engine copies each PSUM score tile into the HIGH fp16 halves of a uint32
tile as fp16(score + 0.5) (always positive, so fp32 bit-pattern order ==
value order), whose LOW uint16 halves hold a persistent column iota.  A
top-8 MAX over those packed words per 512-wide chunk yields (quantized
score, column) pairs — 512 candidates per core per query, no FIND_INDEX
pass, no duplicate-value ambiguity.

The host reduces the 8 partial candidate sets: it rescores each query's top
RESCORE candidates (by packed device score) in fp32 and selects the final
top-256 by (score desc, index asc) — the order jax.lax.top_k uses.  The
device sweep is only a candidate filter: an EPS-wide window below the 256th
device score is rescored, where EPS is ~4x the worst observed
fp16-matmul + fp16-capture deviation, and per-chunk/per-query certificates
fall back to an exact recompute of any query where a margin could bind
(the deepest chunk on this workload holds 6 windowed candidates vs the 8
the device captures; the fallback never triggers in practice).
"""

import numpy as np

import concourse.bacc as bacc
import concourse.mybir as mybir
from concourse.tile import TileContext
from concourse.bass_utils import run_bass_kernel_spmd

N_CORES = 8
MEM_SIZE = 262144
FEAT_DIM = 512
TOP_K = 256
BATCH = 128

SHARD = MEM_SIZE // N_CORES          # 32768 rows per core
N_TILES = SHARD // 512               # 64 tiles of 512 mem rows
CANDS = N_TILES * 8                  # 512 candidates per core per query

RESCORE = 512                        # candidates rescored in fp32 per query
EPS = 2e-3                           # candidate window below the device cut
DEV_BOUND = 1e-3                     # certified max |fp32 - device-capture|

KBUFS = 8                            # key-tile double-buffer depth
SLOTS = 4                            # packed score slots / PSUM banks used

_compiled = None


def _build(kbufs=None, slots=None, dma_alt=False, ramp_split=0):
    kbufs = KBUFS if kbufs is None else kbufs
    slots = SLOTS if slots is None else slots

    f32 = mybir.dt.float32
    f16 = mybir.dt.float16
    u16 = mybir.dt.uint16
    u32 = mybir.dt.uint32

    nc = bacc.Bacc("TRN2", target_bir_lowering=False, debug=False)
    qt_d = nc.dram_tensor("qt", [4, 128, BATCH], f16, kind="ExternalInput")
    kt_d = nc.dram_tensor("kt", [N_TILES, 128, 4 * 512], f16, kind="ExternalInput")
    io_d = nc.dram_tensor("iota16", [128, 512], u16, kind="ExternalInput")
    cv_d = nc.dram_tensor("cand_vals", [BATCH, CANDS], f32, kind="ExternalOutput")

    with TileContext(nc) as tc:
        with (
            tc.tile_pool(name="qpool", bufs=1) as qpool,
            tc.tile_pool(name="kpool", bufs=kbufs) as kpool,
            tc.tile_pool(name="cpool", bufs=1) as cpool,
            tc.tile_pool(name="pspool", bufs=slots, space="PSUM") as pspool,
        ):
            qt_sb = qpool.tile([128, 4, BATCH], f16)
            nc.sync.dma_start(out=qt_sb, in_=qt_d.ap().rearrange("s p b -> p s b"))
            iota_sb = qpool.tile([128, 512], u16)
            nc.sync.dma_start(out=iota_sb, in_=io_d.ap())

            vals_sb = cpool.tile([BATCH, CANDS], f32)
            # packed score slots: uint32 words [fp16(score+0.5) | iota16]
            packed = cpool.tile([BATCH, slots * 512], u32)
            pk_u16 = packed.bitcast(u16)       # [128, slots*1024]
            pk_f16 = packed.bitcast(f16)
            pk_f32 = packed.bitcast(f32)       # [128, slots*512]
            for s in range(slots):
                lo = pk_u16[:, s * 1024:(s + 1) * 1024][:, 0::2]
                nc.vector.tensor_copy(lo, iota_sb)

            for t in range(N_TILES):
                kt_tile = kpool.tile([128, 4 * 512], f16)
                eng = nc.gpsimd if (dma_alt and t % 2) else nc.sync
                if t < ramp_split:
                    for q in range(4):
                        eng.dma_start(
                            out=kt_tile[:, q * 512:(q + 1) * 512],
                            in_=kt_d.ap()[t][:, q * 512:(q + 1) * 512],
                        )
                else:
                    eng.dma_start(out=kt_tile, in_=kt_d.ap()[t])

                ps = pspool.tile([BATCH, 512], f32)
                for s in range(4):
                    nc.tensor.matmul(
                        ps,
                        lhsT=qt_sb[:, s, :],
                        rhs=kt_tile[:, s * 512:(s + 1) * 512],
                        start=(s == 0),
                        stop=(s == 3),
                    )

                sl = t % slots
                hi = pk_f16[:, sl * 1024:(sl + 1) * 1024][:, 1::2]
                nc.scalar.activation(
                    out=hi, in_=ps,
                    func=mybir.ActivationFunctionType.Copy, bias=0.5,
                )
                nc.vector.max(
                    out=vals_sb[:, t * 8:(t + 1) * 8],
                    in_=pk_f32[:, sl * 512:(sl + 1) * 512],
                )

            nc.sync.dma_start(out=cv_d.ap(), in_=vals_sb)

    nc.compile()
    return nc


def _get_compiled():
    global _compiled
    if _compiled is None:
        _compiled = _build()
    return _compiled


def _normalize_q(query):
    q = np.asarray(query, dtype=np.float32)
    return q / np.linalg.norm(q, axis=1, keepdims=True).astype(np.float32)


def _prep_inputs(query, spatial_key):
    """Host-side shard + layout transform + fp16 downcast."""
    qn = _normalize_q(query)
    # [4, 128, BATCH]: d-slice s, d-within-slice p, query b
    qt = np.ascontiguousarray(qn.T.reshape(4, 128, BATCH)).astype(np.float16)
    iota = np.broadcast_to(np.arange(512, dtype=np.uint16), (128, 512)).copy()

    sk = np.asarray(spatial_key, dtype=np.float32)
    in_maps = []
    for c in range(N_CORES):
        shard = sk[c * SHARD:(c + 1) * SHARD]           # [32768, 512]
        kt = shard.T                                     # [512, 32768] (view)
        # [4, 128, N_TILES, 512] -> [N_TILES, 128, 4, 512]
        kt = kt.reshape(4, 128, N_TILES, 512).transpose(2, 1, 0, 3)
        kt = np.ascontiguousarray(kt, dtype=np.float32).astype(np.float16)
        in_maps.append({"qt": qt, "kt": kt.reshape(N_TILES, 128, 4 * 512),
                        "iota16": iota})
    return in_maps


def _merge_host(results, query, spatial_key):
    """Reduce per-core candidate sets to the exact fp32 global top-256."""
    qn = _normalize_q(query)
    sk = np.asarray(spatial_key, dtype=np.float32)

    # unpack [fp16(score+0.5) | column] words; candidate slot j of core c is
    # from tile j//8, so its global row is c*SHARD + (j//8)*512 + column
    tile_base = (np.arange(N_TILES, dtype=np.int64)[:, None] * 512).reshape(1, N_TILES, 1)
    all_vals = np.empty((BATCH, N_CORES * CANDS), dtype=np.float32)
    all_gidx = np.empty((BATCH, N_CORES * CANDS), dtype=np.int64)
    for c in range(N_CORES):
        u = results[c]["cand_vals"].view(np.uint32)           # [128, CANDS]
        col = (u & np.uint32(0xFFFF)).astype(np.int64)
        score = (u >> np.uint32(16)).astype(np.uint16).view(np.float16)
        score = score.astype(np.float32) - 0.5
        gidx = (col.reshape(BATCH, N_TILES, 8) + tile_base + c * SHARD
                ).reshape(BATCH, CANDS)
        all_vals[:, c * CANDS:(c + 1) * CANDS] = score
        all_gidx[:, c * CANDS:(c + 1) * CANDS] = gidx

    # rescore each query's top-RESCORE candidates (by device score) in fp32
    part = np.argpartition(-all_vals, RESCORE - 1, axis=1)[:, :RESCORE]
    rows = np.arange(BATCH)[:, None]
    cand_dev = all_vals[rows, part]                           # [128, RESCORE]
    cand_gidx = all_gidx[rows, part]
    cand_f32 = np.empty_like(cand_dev)
    for b in range(BATCH):
        cand_f32[b] = sk[cand_gidx[b]] @ qn[b]

    # exact top-256 among rescored: (value desc, index asc) == lax.top_k order
    sel = np.argpartition(-cand_f32, TOP_K - 1, axis=1)[:, :TOP_K]
    sel_vals = cand_f32[rows, sel]
    sel_gidx = cand_gidx[rows, sel]
    order = np.lexsort((sel_gidx, -sel_vals), axis=1)
    top_vals = np.take_along_axis(sel_vals, order, axis=1)
    top_gidx = np.take_along_axis(sel_gidx, order, axis=1)

    # --- certificates that the device filter cannot have dropped a member ---
    That = -np.partition(-all_vals, TOP_K - 1, axis=1)[:, TOP_K - 1]
    window_floor = That - EPS                                 # [128]
    # (1) every candidate the window wants must have been rescored
    bad = cand_dev.min(axis=1) > window_floor
    # (2) observed device-vs-fp32 deviation must stay well inside EPS
    dev = np.abs(cand_f32 - cand_dev).max(axis=1)
    bad |= dev > DEV_BOUND
    # (3) the fp32 cut must clear the window floor by the deviation bound
    bad |= top_vals[:, TOP_K - 1] - window_floor < dev
    # (4) no chunk may have had >8 wanted candidates: its 8th capture must
    #     sit below the window floor
    m8 = np.stack([all_vals[:, c * CANDS:(c + 1) * CANDS]
                   .reshape(BATCH, N_TILES, 8)[:, :, 7]
                   for c in range(N_CORES)], axis=1)          # [128, 8, N_TILES]
    bad |= (m8 >= window_floor[:, None, None]).any(axis=(1, 2))

    for b in np.nonzero(bad)[0]:
        srow = sk @ qn[b]
        p = np.argpartition(-srow, TOP_K - 1)[:TOP_K]
        o = np.lexsort((p, -srow[p]))
        top_vals[b] = srow[p][o]
        top_gidx[b] = p[o]

    return top_vals, top_gidx


def kernel(query, spatial_key, color_value):
    nc = _get_compiled()
    in_maps = _prep_inputs(query, spatial_key)
    res = run_bass_kernel_spmd(nc, in_maps, core_ids=list(range(N_CORES)))
    top_vals, top_gidx = _merge_host(res.results, query, spatial_key)

    topk_index = top_gidx.astype(np.int32)
    topk_score = top_vals.astype(np.float32)
    cv = np.asarray(color_value, dtype=np.float32)
    topk_feat = cv[top_gidx]
    return topk_feat, topk_score, topk_index


# revision 20
# speedup vs baseline: 1.5865x; 1.5865x over previous
"""Trainium2 Bass kernel for nn_Memory_Network (retrieval kNN).

reference semantics:
    q = query / ||query||_2                  [128, 512]
    scores = q @ spatial_key.T               [128, 262144]
    topk_score, topk_index = top_k(scores, 256)
    topk_feat = color_value[topk_index]      [128, 256, 512]
    returns (topk_feat, topk_score, topk_index)

Sharding: the memory bank is split along mem_size across 8 cores (32768 rows
each).  Each core streams its host-transposed key shard — downcast to
fp8-e4m3 so the HBM stream halves to 16MB/core — through the PE with fp16
queries stationary; scores accumulate in fp32 PSUM.

Candidate extraction packs the column index into the score's low mantissa
bits so a single DVE MAX both selects and identifies candidates: the ACT
engine copies each PSUM score tile into the HIGH fp16 halves of a uint32
tile as fp16(score + 0.5) (always positive, so fp32 bit-pattern order ==
value order), whose LOW uint16 halves hold a persistent column iota.  A
top-8 MAX over those packed words per 512-wide chunk yields (quantized
score, column) pairs — 512 candidates per core per query, no FIND_INDEX
pass, no duplicate-value ambiguity.

The host reduces the 8 partial candidate sets: it rescores each query's top
RESCORE candidates (by packed device score) in fp32 and selects the final
top-256 by (score desc, index asc) — the order jax.lax.top_k uses.  The
device sweep is only a candidate filter: an EPS-wide window below the 256th
device score is rescored, where EPS covers the worst observed
fp8-matmul + fp16-capture deviation with margin, and per-chunk/per-query
certificates fall back to an exact recompute of any query where a margin
could bind (on this workload the capture loses no true member and at most
one query gets flagged; the fallback keeps the output exact regardless).
"""

import numpy as np
import ml_dtypes

import concourse.bacc as bacc
import concourse.mybir as mybir
from concourse.tile import TileContext
from concourse.bass_utils import run_bass_kernel_spmd

N_CORES = 8
MEM_SIZE = 262144
FEAT_DIM = 512
TOP_K = 256
BATCH = 128

SHARD = MEM_SIZE // N_CORES          # 32768 rows per core
N_TILES = SHARD // 512               # 64 tiles of 512 mem rows
CANDS = N_TILES * 8                  # 512 candidates per core per query

RESCORE = 1024                       # candidates rescored in fp32 per query
EPS = 1.2e-2                         # candidate window below the device cut
DEV_BOUND = 9e-3                     # certified max |fp32 - device-capture|

KBUFS = 8                            # key-tile double-buffer depth
SLOTS = 4                            # packed score slots / PSUM banks used

_compiled = None


def _build(kbufs=None, slots=None, dma_alt=False, ramp_split=0):
    kbufs = KBUFS if kbufs is None else kbufs
    slots = SLOTS if slots is None else slots

    f32 = mybir.dt.float32
    f16 = mybir.dt.float16
    u16 = mybir.dt.uint16
    u32 = mybir.dt.uint32

    nc = bacc.Bacc("TRN2", target_bir_lowering=False, debug=False)
    qt_d = nc.dram_tensor("qt", [4, 128, BATCH], f16, kind="ExternalInput")
    f8 = mybir.dt.float8e4
    kt_d = nc.dram_tensor("kt", [N_TILES, 128, 4 * 512], f8, kind="ExternalInput")
    io_d = nc.dram_tensor("iota16", [128, 512], u16, kind="ExternalInput")
    cv_d = nc.dram_tensor("cand_vals", [BATCH, CANDS], f32, kind="ExternalOutput")

    with TileContext(nc) as tc:
        with (
            tc.tile_pool(name="qpool", bufs=1) as qpool,
            tc.tile_pool(name="kpool", bufs=kbufs) as kpool,
            tc.tile_pool(name="cpool", bufs=1) as cpool,
            tc.tile_pool(name="pspool", bufs=slots, space="PSUM") as pspool,
        ):
            qt_sb = qpool.tile([128, 4, BATCH], f16)
            nc.sync.dma_start(out=qt_sb, in_=qt_d.ap().rearrange("s p b -> p s b"))
            iota_sb = qpool.tile([128, 512], u16)
            nc.sync.dma_start(out=iota_sb, in_=io_d.ap())

            vals_sb = cpool.tile([BATCH, CANDS], f32)
            # packed score slots: uint32 words [fp16(score+0.5) | iota16]
            packed = cpool.tile([BATCH, slots * 512], u32)
            pk_u16 = packed.bitcast(u16)       # [128, slots*1024]
            pk_f16 = packed.bitcast(f16)
            pk_f32 = packed.bitcast(f32)       # [128, slots*512]
            for s in range(slots):
                lo = pk_u16[:, s * 1024:(s + 1) * 1024][:, 0::2]
                nc.vector.tensor_copy(lo, iota_sb)

            for t in range(N_TILES):
                kt_tile = kpool.tile([128, 4 * 512], mybir.dt.float8e4)
                eng = nc.gpsimd if (dma_alt and t % 2) else nc.sync
                if t < ramp_split:
                    for q in range(4):
                        eng.dma_start(
                            out=kt_tile[:, q * 512:(q + 1) * 512],
                            in_=kt_d.ap()[t][:, q * 512:(q + 1) * 512],
                        )
                else:
                    eng.dma_start(out=kt_tile, in_=kt_d.ap()[t])

                ps = pspool.tile([BATCH, 512], f32)
                for s in range(4):
                    nc.tensor.matmul(
                        ps,
                        lhsT=qt_sb[:, s, :],
                        rhs=kt_tile[:, s * 512:(s + 1) * 512],
                        start=(s == 0),
                        stop=(s == 3),
                    )

                sl = t % slots
                hi = pk_f16[:, sl * 1024:(sl + 1) * 1024][:, 1::2]
                nc.scalar.activation(
                    out=hi, in_=ps,
                    func=mybir.ActivationFunctionType.Copy, bias=0.5,
                )
                nc.vector.max(
                    out=vals_sb[:, t * 8:(t + 1) * 8],
                    in_=pk_f32[:, sl * 512:(sl + 1) * 512],
                )

            nc.sync.dma_start(out=cv_d.ap(), in_=vals_sb)

    nc.compile()
    return nc


def _get_compiled():
    global _compiled
    if _compiled is None:
        _compiled = _build()
    return _compiled


def _normalize_q(query):
    q = np.asarray(query, dtype=np.float32)
    return q / np.linalg.norm(q, axis=1, keepdims=True).astype(np.float32)


def _prep_inputs(query, spatial_key):
    """Host-side shard + layout transform + fp16 downcast."""
    qn = _normalize_q(query)
    # [4, 128, BATCH]: d-slice s, d-within-slice p, query b
    qt = np.ascontiguousarray(qn.T.reshape(4, 128, BATCH)).astype(np.float16)
    iota = np.broadcast_to(np.arange(512, dtype=np.uint16), (128, 512)).copy()

    sk = np.asarray(spatial_key, dtype=np.float32)
    in_maps = []
    for c in range(N_CORES):
        shard = sk[c * SHARD:(c + 1) * SHARD]           # [32768, 512]
        kt = shard.T                                     # [512, 32768] (view)
        # [4, 128, N_TILES, 512] -> [N_TILES, 128, 4, 512]
        kt = kt.reshape(4, 128, N_TILES, 512).transpose(2, 1, 0, 3)
        kt = np.ascontiguousarray(kt, dtype=np.float32).astype(ml_dtypes.float8_e4m3)
        in_maps.append({"qt": qt, "kt": kt.reshape(N_TILES, 128, 4 * 512),
                        "iota16": iota})
    return in_maps


def _merge_host(results, query, spatial_key):
    """Reduce per-core candidate sets to the exact fp32 global top-256."""
    qn = _normalize_q(query)
    sk = np.asarray(spatial_key, dtype=np.float32)

    # unpack [fp16(score+0.5) | column] words; candidate slot j of core c is
    # from tile j//8, so its global row is c*SHARD + (j//8)*512 + column
    tile_base = (np.arange(N_TILES, dtype=np.int64)[:, None] * 512).reshape(1, N_TILES, 1)
    all_vals = np.empty((BATCH, N_CORES * CANDS), dtype=np.float32)
    all_gidx = np.empty((BATCH, N_CORES * CANDS), dtype=np.int64)
    for c in range(N_CORES):
        u = results[c]["cand_vals"].view(np.uint32)           # [128, CANDS]
        col = (u & np.uint32(0xFFFF)).astype(np.int64)
        score = (u >> np.uint32(16)).astype(np.uint16).view(np.float16)
        score = score.astype(np.float32) - 0.5
        gidx = (col.reshape(BATCH, N_TILES, 8) + tile_base + c * SHARD
                ).reshape(BATCH, CANDS)
        all_vals[:, c * CANDS:(c + 1) * CANDS] = score
        all_gidx[:, c * CANDS:(c + 1) * CANDS] = gidx

    # rescore each query's top-RESCORE candidates (by device score) in fp32
    part = np.argpartition(-all_vals, RESCORE - 1, axis=1)[:, :RESCORE]
    rows = np.arange(BATCH)[:, None]
    cand_dev = all_vals[rows, part]                           # [128, RESCORE]
    cand_gidx = all_gidx[rows, part]
    cand_f32 = np.empty_like(cand_dev)
    for b in range(BATCH):
        cand_f32[b] = sk[cand_gidx[b]] @ qn[b]

    # exact top-256 among rescored: (value desc, index asc) == lax.top_k order
    sel = np.argpartition(-cand_f32, TOP_K - 1, axis=1)[:, :TOP_K]
    sel_vals = cand_f32[rows, sel]
    sel_gidx = cand_gidx[rows, sel]
    order = np.lexsort((sel_gidx, -sel_vals), axis=1)
    top_vals = np.take_along_axis(sel_vals, order, axis=1)
    top_gidx = np.take_along_axis(sel_gidx, order, axis=1)

    # --- certificates that the device filter cannot have dropped a member ---
    That = -np.partition(-all_vals, TOP_K - 1, axis=1)[:, TOP_K - 1]
    window_floor = That - EPS                                 # [128]
    # (1) every candidate the window wants must have been rescored
    bad = cand_dev.min(axis=1) > window_floor
    # (2) observed device-vs-fp32 deviation must stay well inside EPS
    dev = np.abs(cand_f32 - cand_dev).max(axis=1)
    bad |= dev > DEV_BOUND
    # (3) the fp32 cut must clear the window floor by the deviation bound
    bad |= top_vals[:, TOP_K - 1] - window_floor < dev
    # (4) no chunk may have had >8 wanted candidates: its 8th capture must
    #     sit below the window floor
    m8 = np.stack([all_vals[:, c * CANDS:(c + 1) * CANDS]
                   .reshape(BATCH, N_TILES, 8)[:, :, 7]
                   for c in range(N_CORES)], axis=1)          # [128, 8, N_TILES]
    bad |= (m8 >= window_floor[:, None, None]).any(axis=(1, 2))

    for b in np.nonzero(bad)[0]:
        srow = sk @ qn[b]
        p = np.argpartition(-srow, TOP_K - 1)[:TOP_K]
        o = np.lexsort((p, -srow[p]))
        top_vals[b] = srow[p][o]
        top_gidx[b] = p[o]

    return top_vals, top_gidx


def kernel(query, spatial_key, color_value):
    nc = _get_compiled()
    in_maps = _prep_inputs(query, spatial_key)
    res = run_bass_kernel_spmd(nc, in_maps, core_ids=list(range(N_CORES)))
    top_vals, top_gidx = _merge_host(res.results, query, spatial_key)

    topk_index = top_gidx.astype(np.int32)
    topk_score = top_vals.astype(np.float32)
    cv = np.asarray(color_value, dtype=np.float32)
    topk_feat = cv[top_gidx]
    return topk_feat, topk_score, topk_index
